# revision 18
# baseline (speedup 1.0000x reference)
"""Fused 2-layer GAT block for Trainium2, SPMD over 8 NeuronCores.

Problem shapes (hardcoded): B=2, N=2048, D=512, H1=8 (Dh=64), H2=1 (Dh=512).

Strategy
--------
Two SPMD launches (layer 2 depends on the full layer-1 output):

Launch 1 (layer 1), core = (batch b, head-pair hh):  b = core//4, hh = core%4.
  Each core computes LN(x[b]) @ W1 (full, redundant within a batch group),
  then flash-style attention for its 2 heads over all 2048 queries:
  scores are built directly in transposed layout [128 keys_j, 2048 queries_i]
  (so the attn @ h matmul needs no per-tile transposes), the softmax
  denominator comes free from a ones-column appended to h (lhsT), and
  softmax normalization is folded into the output epilogue.
Launch 2 (layer 2), core = (batch b, query-chunk qc): 512 query rows/core,
  single head with Dh=512; same transposed-score scheme with 4+1 lhsT chunks.

Host-side weight folding: LN affine (g,b) folds into W' = diag(g)@W and
c = b@W; the attention projections a_src/a_dst fold into extra projection
columns wa = W'@a (so s,t come out of the projection matmul for free); the
constant c shifts s,t by scalars (passed as a tiny input) and the attention
output by +c (folded into the residual input on host).
"""

import os
import numpy as np

import concourse.bass as bass
import concourse.bacc as bacc
import concourse.mybir as mybir
import concourse.tile as tile
from concourse.bass_utils import run_bass_kernel_spmd
from concourse.masks import make_identity

F32 = mybir.dt.float32
BF16 = mybir.dt.bfloat16
AF = mybir.ActivationFunctionType
ALU = mybir.AluOpType

ALPHA = 0.2
EPS = 1e-6
P = 128
N = 2048
D = 512
NT = N // P          # 16 row tiles
KB = D // P          # 4 contraction blocks
JB = N // P          # 16 key blocks

# every 3rd score tile uses the Prelu (ScalarE) leaky path to balance
# engine load; the rest use the DVE max(u, alpha*u) path.
PRELU_EVERY = int(os.environ.get("GAT_PRELU_EVERY", "3"))

LAST_EXEC_NS = {}


def _install_ntff_hook():
    """Wire up NTFF profiling under axon when antenv.axon_hooks is absent."""
    import sys
    import types
    try:
        from antenv.axon_hooks import get_axon_ntff_profile_hook  # noqa: F401
        return
    except ImportError:
        pass
    try:
        from trn_agent_boot.trn_boot import _ntff_profile_via_ctypes
        hook = _ntff_profile_via_ctypes("/opt/axon/libaxon_pjrt.so")
    except Exception:
        hook = None
    mod = types.ModuleType("antenv.axon_hooks")
    mod.get_axon_ntff_profile_hook = lambda: hook
    mod.set_axon_ntff_profile_hook = lambda h: None
    sys.modules["antenv.axon_hooks"] = mod
    import antenv
    antenv.axon_hooks = mod
    import concourse.bass_utils as bu
    bu.upload_artifacts = lambda tmpdir: tmpdir


def _bcast_ap(row_ap, parts=P):
    """DRAM row -> broadcast-over-partitions AP (partition stride 0)."""
    return bass.AP(
        tensor=row_ap.tensor,
        offset=row_ap.offset,
        ap=[[0, parts]] + [list(d) for d in row_ap.ap],
    )


def _ln_tile(nc, work, eps_t, x_ap, out_ap):
    """LayerNorm one [128, D] tile (fp32 stats) -> bf16 normalized tile."""
    stats = work.tile([P, 6], F32, tag="stats")
    mv = work.tile([P, 2], F32, tag="mv")
    nc.vector.bn_stats(stats, x_ap)
    nc.vector.bn_aggr(mv, stats)
    std = work.tile([P, 1], F32, tag="std")
    nc.scalar.activation(std, mv[:, 1:2], AF.Sqrt, bias=eps_t)
    rstd = work.tile([P, 1], F32, tag="rstd")
    nc.vector.reciprocal(rstd, std)
    nc.vector.tensor_scalar(out_ap, x_ap, mv[:, 0:1], rstd, ALU.subtract, ALU.mult)


def _score_tile(nc, spool, sbcast_ap, t_col, at_col, biasT_ap, nfree, use_prelu):
    """exp(leaky(s_i + t_j) + bias_ji) as a [128, nfree] bf16 tile."""
    e = spool.tile([P, nfree], BF16, tag="s1")
    if use_prelu:
        l = spool.tile([P, nfree], BF16, tag="s3")
        nc.scalar.activation(l, sbcast_ap, AF.Prelu, bias=t_col, scale=1.0,
                             alpha=ALPHA)
        nc.vector.tensor_tensor(e, l, biasT_ap, ALU.add)
    else:
        u = spool.tile([P, nfree], BF16, tag="s1")
        nc.vector.tensor_scalar(u, sbcast_ap, t_col, None, ALU.add)
        v = spool.tile([P, nfree], BF16, tag="s2")
        nc.vector.tensor_scalar(v, sbcast_ap, ALPHA, at_col, ALU.mult, ALU.add)
        l = spool.tile([P, nfree], BF16, tag="s3")
        nc.vector.tensor_tensor(l, u, v, ALU.max)
        nc.vector.tensor_tensor(e, l, biasT_ap, ALU.add)
    p = spool.tile([P, nfree], BF16, tag="s2")
    nc.scalar.activation(p, e, AF.Exp)
    return p


def build_l1():
    """Layer-1 program: per core, 2 heads x 2048 queries x 2048 keys."""
    nc = bacc.Bacc("TRN2", target_bir_lowering=False, debug=False, num_devices=8)
    xd = nc.declare_dram_parameter("x", [N, D], F32, isOutput=False)
    xrd = nc.declare_dram_parameter("xres", [N, 2 * 64], F32, isOutput=False)
    bd = nc.declare_dram_parameter("biasb", [N, N], F32, isOutput=False)
    wd = nc.declare_dram_parameter("wproj", [D, 132], BF16, isOutput=False)
    sctd = nc.declare_dram_parameter("sct", [1, 2], F32, isOutput=False)
    od = nc.declare_dram_parameter("outcols", [N, 128], F32, isOutput=True)

    with tile.TileContext(nc) as tc:
        with (
            tc.tile_pool(name="const", bufs=1) as const,
            tc.tile_pool(name="big", bufs=1) as big,
            tc.tile_pool(name="work", bufs=4) as work,
        ):
            id_bf = const.tile([P, P], BF16)
            make_identity(nc, id_bf)
            id_f32 = const.tile([P, P], F32)
            make_identity(nc, id_f32)
            eps_t = const.tile([P, 1], F32)
            nc.vector.memset(eps_t, EPS)
            w_sb = const.tile([P, KB, 132], BF16)
            nc.sync.dma_start(out=w_sb,
                              in_=wd[:].rearrange("(kb p) c -> p kb c", p=P))
            sct_sb = const.tile([P, 2], F32)
            nc.gpsimd.dma_start(out=sct_sb, in_=_bcast_ap(sctd[0, :]))

            x_sb = big.tile([P, NT, D], F32)
            nc.sync.dma_start(out=x_sb,
                              in_=xd[:].rearrange("(nt p) d -> p nt d", p=P))
            xres_sb = big.tile([P, NT, 128], F32)
            nc.sync.dma_start(out=xres_sb,
                              in_=xrd[:].rearrange("(nt p) d -> p nt d", p=P))

            h_aug = big.tile([P, NT, 2, 65], BF16)
            nc.vector.memset(h_aug[:, :, :, 64:65], 1.0)
            st = big.tile([P, NT, 4], F32)       # proj cols: src0 src1 dst0 dst1
            t_adj = big.tile([P, NT, 2], F32)    # t + (s_c + t_c), per head
            at_adj = big.tile([P, NT, 2], F32)   # alpha * t_adj
            biasT = big.tile([P, JB, N], BF16)
            sbcast = big.tile([P, 2, N], BF16)
            out_stage = big.tile([P, NT, 128], F32)

            # ---- LN + transpose + projection (h, s, t) ----
            with tc.tile_pool(name="xn_pool", bufs=1) as xnp:
                xn_all = xnp.tile([P, NT, D], BF16)
                for nt in range(NT):
                    _ln_tile(nc, work, eps_t, x_sb[:, nt], xn_all[:, nt])

                xnT = xnp.tile([P, KB, N], BF16)
                with tc.tile_pool(name="ptr1", bufs=2, space="PSUM") as ptr1:
                    for kb in range(KB):
                        pt = ptr1.tile([P, N], BF16, tag="pt1")
                        for nt in range(NT):
                            nc.tensor.transpose(
                                pt[:, nt * P:(nt + 1) * P],
                                xn_all[:, nt, kb * P:(kb + 1) * P], id_bf)
                        if kb % 2 == 0:
                            nc.vector.tensor_copy(out=xnT[:, kb], in_=pt)
                        else:
                            nc.scalar.copy(xnT[:, kb], pt)

                with tc.tile_pool(name="pproj", bufs=4, space="PSUM") as pp:
                    for nt in range(NT):
                        pt = pp.tile([P, 132], F32, tag="pp")
                        for kb in range(KB):
                            nc.tensor.matmul(
                                pt, xnT[:, kb, nt * P:(nt + 1) * P],
                                w_sb[:, kb], start=(kb == 0), stop=(kb == KB - 1))
                        nc.vector.tensor_copy(
                            out=h_aug[:, nt, :, 0:64],
                            in_=pt[:, 0:128].rearrange("p (h k) -> p h k", h=2))
                        nc.vector.tensor_copy(out=st[:, nt], in_=pt[:, 128:132])

            # ---- t columns (+ const shift), alpha*t ----
            for hi in range(2):
                nc.vector.tensor_scalar(t_adj[:, :, hi], st[:, :, 2 + hi],
                                        sct_sb[:, hi:hi + 1], None, ALU.add)
            nc.vector.tensor_scalar(at_adj, t_adj, ALPHA, None, ALU.mult)

            # ---- s broadcast rows: [128,16] col-tile -> dram row -> bcast ----
            with (
                tc.tile_pool(name="sdram", bufs=1, space="DRAM") as sdram,
                tc.tile_pool(name="ptrs", bufs=2, space="PSUM") as ptrs,
            ):
                s_dram = sdram.tile([2, N], BF16)
                for hi in range(2):
                    pts = ptrs.tile([NT, P], F32, tag="pts")
                    nc.tensor.transpose(pts, st[:, :, hi], id_f32)
                    sfl = work.tile([NT, P], BF16, tag="sfl")
                    # add the constant shift for s while copying out of PSUM
                    nc.scalar.activation(sfl, pts, AF.Copy)
                    nc.sync.dma_start(
                        out=s_dram[hi].rearrange("(a b) -> a b", b=P), in_=sfl)
                for hi in range(2):
                    nc.gpsimd.dma_start(out=sbcast[:, hi],
                                        in_=_bcast_ap(s_dram[hi]))

            # ---- bias transpose: [2048 i, 2048 j] -> bf16 [j, i] tiles ----
            with (
                tc.tile_pool(name="braw", bufs=2) as brp,
                tc.tile_pool(name="ptr2", bufs=2, space="PSUM") as ptr2,
            ):
                for jb in range(JB):
                    braw = brp.tile([P, NT, P], F32, tag="braw")
                    nc.sync.dma_start(
                        out=braw,
                        in_=bd[:, jb * P:(jb + 1) * P]
                        .rearrange("(nt p) j -> p nt j", p=P))
                    pt = ptr2.tile([P, N], F32, tag="pt2")
                    for c in range(NT):
                        nc.tensor.transpose(pt[:, c * P:(c + 1) * P],
                                            braw[:, c], id_f32)
                    if jb % 2 == 0:
                        nc.vector.tensor_copy(out=biasT[:, jb], in_=pt)
                    else:
                        nc.scalar.copy(biasT[:, jb], pt)

            # ---- scores + attention matmul + epilogue ----
            with (
                tc.tile_pool(name="spool", bufs=2) as spool,
                tc.tile_pool(name="evp", bufs=2) as evp,
                tc.tile_pool(name="pacc", bufs=1, space="PSUM") as pacc,
                tc.tile_pool(name="ptr3", bufs=1, space="PSUM") as ptr3,
            ):
                for hi in range(2):
                    acc = pacc.tile([65, N], F32, tag="acc")
                    for jb in range(JB):
                        p = _score_tile(
                            nc, spool, sbcast[:, hi],
                            t_adj[:, jb, hi:hi + 1], at_adj[:, jb, hi:hi + 1],
                            biasT[:, jb], N,
                            use_prelu=(jb % PRELU_EVERY == PRELU_EVERY - 1))
                        for ih in range(4):
                            sl = slice(ih * 512, (ih + 1) * 512)
                            nc.tensor.matmul(acc[:, sl], h_aug[:, jb, hi],
                                             p[:, sl], start=(jb == 0),
                                             stop=(jb == JB - 1))
                    # epilogue: normalize + transpose back + residual
                    ev = evp.tile([65, N], F32, tag="ev")
                    nc.vector.tensor_copy(out=ev, in_=acc)
                    ptro = ptr3.tile([P, NT, P], F32, tag="ptro")
                    for nt in range(NT):
                        nc.tensor.transpose(ptro[:, nt, 0:65],
                                            ev[:, nt * P:(nt + 1) * P],
                                            id_f32[0:65, 0:65])
                    ov = evp.tile([P, NT, 65], F32, tag="ov")
                    nc.scalar.copy(ov, ptro[:, :, 0:65])
                    for nt in range(NT):
                        rz = work.tile([P, 1], F32, tag="rz")
                        nc.vector.reciprocal(rz, ov[:, nt, 64:65])
                        sc = work.tile([P, 64], F32, tag="sc")
                        nc.vector.tensor_scalar(sc, ov[:, nt, 0:64], rz, None,
                                                ALU.mult)
                        nc.vector.tensor_tensor(
                            out_stage[:, nt, hi * 64:(hi + 1) * 64], sc,
                            xres_sb[:, nt, hi * 64:(hi + 1) * 64], ALU.add)

            nc.sync.dma_start(
                out=od[:].rearrange("(nt p) c -> p nt c", p=P), in_=out_stage)

    nc.compile()
    return nc


def build_l2():
    """Layer-2 program: per core, 1 head x 512 queries x 2048 keys."""
    nc = bacc.Bacc("TRN2", target_bir_lowering=False, debug=False, num_devices=8)
    ad = nc.declare_dram_parameter("attnb", [N, D], F32, isOutput=False)
    rd = nc.declare_dram_parameter("resid", [512, D], F32, isOutput=False)
    bd = nc.declare_dram_parameter("biasc", [512, N], F32, isOutput=False)
    wd = nc.declare_dram_parameter("wproj2", [D, 514], BF16, isOutput=False)
    sctd = nc.declare_dram_parameter("sct2", [1, 1], F32, isOutput=False)
    od = nc.declare_dram_parameter("outc", [512, D], F32, isOutput=True)

    IQ = 512            # queries per core
    QT = IQ // P        # 4 query tiles

    with tile.TileContext(nc) as tc:
        with (
            tc.tile_pool(name="const", bufs=1) as const,
            tc.tile_pool(name="big", bufs=1) as big,
            tc.tile_pool(name="work", bufs=4) as work,
        ):
            id_bf = const.tile([P, P], BF16)
            make_identity(nc, id_bf)
            id_f32 = const.tile([P, P], F32)
            make_identity(nc, id_f32)
            eps_t = const.tile([P, 1], F32)
            nc.vector.memset(eps_t, EPS)
            ones_col = const.tile([P, 1], BF16)
            nc.vector.memset(ones_col, 1.0)
            w_sb = const.tile([P, KB, 514], BF16)
            nc.sync.dma_start(out=w_sb,
                              in_=wd[:].rearrange("(kb p) c -> p kb c", p=P))
            sct_sb = const.tile([P, 1], F32)
            nc.gpsimd.dma_start(out=sct_sb, in_=_bcast_ap(sctd[0, :]))

            a_sb = big.tile([P, NT, D], F32)
            nc.sync.dma_start(out=a_sb,
                              in_=ad[:].rearrange("(nt p) d -> p nt d", p=P))
            r_sb = big.tile([P, QT, D], F32)
            nc.sync.dma_start(out=r_sb,
                              in_=rd[:].rearrange("(nt p) d -> p nt d", p=P))

            h2 = big.tile([P, NT, D], BF16)
            sq = big.tile([P, QT], F32)
            t_adj = big.tile([P, NT, 1], F32)
            at_adj = big.tile([P, NT, 1], F32)
            biasT = big.tile([P, JB, IQ], BF16)
            sbcast = big.tile([P, IQ], BF16)
            out_stage = big.tile([P, QT, D], F32)

            # ---- LN (full batch for keys, query chunk for s) + proj ----
            with tc.tile_pool(name="xn_pool", bufs=1) as xnp:
                xn_all = xnp.tile([P, NT, D], BF16)
                for nt in range(NT):
                    _ln_tile(nc, work, eps_t, a_sb[:, nt], xn_all[:, nt])
                xnq = xnp.tile([P, QT, D], BF16)
                for nt in range(QT):
                    _ln_tile(nc, work, eps_t, r_sb[:, nt], xnq[:, nt])

                xnT = xnp.tile([P, KB, N], BF16)
                xnqT = xnp.tile([P, KB, IQ], BF16)
                with tc.tile_pool(name="ptr1", bufs=2, space="PSUM") as ptr1:
                    for kb in range(KB):
                        pt = ptr1.tile([P, N], BF16, tag="pt1")
                        for nt in range(NT):
                            nc.tensor.transpose(
                                pt[:, nt * P:(nt + 1) * P],
                                xn_all[:, nt, kb * P:(kb + 1) * P], id_bf)
                        if kb % 2 == 0:
                            nc.vector.tensor_copy(out=xnT[:, kb], in_=pt)
                        else:
                            nc.scalar.copy(xnT[:, kb], pt)
                    for kb in range(KB):
                        ptq = ptr1.tile([P, IQ], BF16, tag="ptq")
                        for nt in range(QT):
                            nc.tensor.transpose(
                                ptq[:, nt * P:(nt + 1) * P],
                                xnq[:, nt, kb * P:(kb + 1) * P], id_bf)
                        nc.vector.tensor_copy(out=xnqT[:, kb], in_=ptq)

                with tc.tile_pool(name="pproj", bufs=2, space="PSUM") as pp:
                    for nt in range(NT):
                        pt = pp.tile([P, D], F32, tag="pp")
                        ptst = pp.tile([P, 2], F32, tag="ppst")
                        for kb in range(KB):
                            lhsT = xnT[:, kb, nt * P:(nt + 1) * P]
                            nc.tensor.matmul(
                                pt, lhsT, w_sb[:, kb, 0:D],
                                start=(kb == 0), stop=(kb == KB - 1))
                            nc.tensor.matmul(
                                ptst, lhsT, w_sb[:, kb, D:D + 2],
                                start=(kb == 0), stop=(kb == KB - 1))
                        nc.vector.tensor_copy(out=h2[:, nt], in_=pt)
                        nc.vector.tensor_scalar(t_adj[:, nt], ptst[:, 1:2],
                                                sct_sb[:, 0:1], None, ALU.add)
                    # s for the query chunk only
                    for nt in range(QT):
                        ptq = pp.tile([P, 2], F32, tag="ppq")
                        for kb in range(KB):
                            nc.tensor.matmul(
                                ptq, xnqT[:, kb, nt * P:(nt + 1) * P],
                                w_sb[:, kb, 512:514], start=(kb == 0),
                                stop=(kb == KB - 1))
                        # s column for the query chunk
                        nc.vector.tensor_copy(out=sq[:, nt:nt + 1],
                                              in_=ptq[:, 0:1])

            nc.vector.tensor_scalar(at_adj, t_adj, ALPHA, None, ALU.mult)

            # ---- s broadcast row ----
            with (
                tc.tile_pool(name="sdram", bufs=1, space="DRAM") as sdram,
                tc.tile_pool(name="ptrs", bufs=1, space="PSUM") as ptrs,
            ):
                s_dram = sdram.tile([1, IQ], BF16)
                pts = ptrs.tile([QT, P], F32, tag="pts")
                nc.tensor.transpose(pts, sq, id_f32)
                sfl = work.tile([QT, P], BF16, tag="sfl")
                nc.scalar.copy(sfl, pts)
                nc.sync.dma_start(
                    out=s_dram[0].rearrange("(a b) -> a b", b=P), in_=sfl)
                nc.gpsimd.dma_start(out=sbcast, in_=_bcast_ap(s_dram[0]))

            # ---- bias transpose ----
            with (
                tc.tile_pool(name="braw", bufs=2) as brp,
                tc.tile_pool(name="ptr2", bufs=2, space="PSUM") as ptr2,
            ):
                for jb in range(JB):
                    braw = brp.tile([P, QT, P], F32, tag="braw")
                    nc.sync.dma_start(
                        out=braw,
                        in_=bd[:, jb * P:(jb + 1) * P]
                        .rearrange("(nt p) j -> p nt j", p=P))
                    pt = ptr2.tile([P, IQ], F32, tag="pt2")
                    for c in range(QT):
                        nc.tensor.transpose(pt[:, c * P:(c + 1) * P],
                                            braw[:, c], id_f32)
                    if jb % 2 == 0:
                        nc.vector.tensor_copy(out=biasT[:, jb], in_=pt)
                    else:
                        nc.scalar.copy(biasT[:, jb], pt)

            # ---- scores + attention matmul ----
            with (
                tc.tile_pool(name="spool", bufs=3) as spool,
                tc.tile_pool(name="pacc", bufs=1, space="PSUM") as pacc,
                tc.tile_pool(name="ptr3", bufs=1, space="PSUM") as ptr3,
            ):
                accs = [pacc.tile([P, IQ], F32, tag=f"acc{m}", name=f"acc{m}")
                        for m in range(4)]
                accz = pacc.tile([1, IQ], F32, tag="accz")
                for jb in range(JB):
                    p = _score_tile(
                        nc, spool, sbcast, t_adj[:, jb, 0:1], at_adj[:, jb, 0:1],
                        biasT[:, jb], IQ,
                        use_prelu=(jb % PRELU_EVERY == PRELU_EVERY - 1))
                    for m in range(4):
                        nc.tensor.matmul(accs[m], h2[:, jb, m * P:(m + 1) * P],
                                         p, start=(jb == 0), stop=(jb == JB - 1))
                    nc.tensor.matmul(accz, ones_col, p, start=(jb == 0),
                                     stop=(jb == JB - 1))

                # ---- epilogue: transpose back, normalize, residual ----
                evz = work.tile([1, IQ], F32, tag="evz")
                nc.vector.tensor_copy(out=evz, in_=accz)
                zt = work.tile([P, QT], F32, tag="zt")
                # z row -> per-partition columns via tiny DMA round trip
                with tc.tile_pool(name="zdram", bufs=1, space="DRAM") as zdram:
                    z_dram = zdram.tile([1, IQ], F32)
                    nc.sync.dma_start(out=z_dram[0:1, :], in_=evz)
                    nc.gpsimd.dma_start(
                        out=zt, in_=z_dram[0].rearrange("(a b) -> b a", b=P))
                rz = work.tile([P, QT], F32, tag="rz")
                nc.vector.reciprocal(rz, zt)

                for m in range(4):
                    ev = work.tile([P, IQ], F32, tag="ev")
                    nc.vector.tensor_copy(out=ev, in_=accs[m])
                    ptro = ptr3.tile([P, QT * P], F32, tag="ptro")
                    for nt in range(QT):
                        nc.tensor.transpose(ptro[:, nt * P:(nt + 1) * P],
                                            ev[:, nt * P:(nt + 1) * P], id_f32)
                    for nt in range(QT):
                        sc = work.tile([P, P], F32, tag="sc")
                        nc.vector.tensor_scalar(sc, ptro[:, nt * P:(nt + 1) * P],
                                                rz[:, nt:nt + 1], None, ALU.mult)
                        nc.vector.tensor_tensor(
                            out_stage[:, nt, m * P:(m + 1) * P], sc,
                            r_sb[:, nt, m * P:(m + 1) * P], ALU.add)

            nc.sync.dma_start(
                out=od[:].rearrange("(nt p) c -> p nt c", p=P), in_=out_stage)

    nc.compile()
    return nc


_CACHE = {}


def _get_programs():
    if "l1" not in _CACHE:
        _CACHE["l1"] = build_l1()
        _CACHE["l2"] = build_l2()
    return _CACHE["l1"], _CACHE["l2"]


def kernel(x, bias, W1, a_src1, a_dst1, g1, b1, W2, a_src2, a_dst2, g2, b2):
    x = np.asarray(x, np.float32)
    bias = np.asarray(bias, np.float32)
    bf = mybir.dt.np(BF16)
    trace = bool(os.environ.get("GAT_TRACE"))
    if trace:
        _install_ntff_hook()

    H, Dh = 8, 64
    # ---- host weight folding, layer 1 ----
    W1g = (np.asarray(g1, np.float32)[:, None] * np.asarray(W1, np.float32))
    c1 = np.asarray(b1, np.float32) @ np.asarray(W1, np.float32)      # [D]
    wa_s1 = np.einsum("dhk,hk->dh", W1g.reshape(D, H, Dh),
                      np.asarray(a_src1, np.float32))                  # [D, 8]
    wa_d1 = np.einsum("dhk,hk->dh", W1g.reshape(D, H, Dh),
                      np.asarray(a_dst1, np.float32))
    c1h = c1.reshape(H, Dh)
    s_c1 = (c1h * np.asarray(a_src1, np.float32)).sum(1)               # [8]
    t_c1 = (c1h * np.asarray(a_dst1, np.float32)).sum(1)

    l1, l2 = _get_programs()

    in_maps = []
    for core in range(8):
        b, hh = core // 4, core % 4
        cols = slice(hh * 128, (hh + 1) * 128)
        heads = [2 * hh, 2 * hh + 1]
        wproj = np.concatenate(
            [W1g[:, cols], wa_s1[:, heads], wa_d1[:, heads]], axis=1)
        sct = np.array([[s_c1[h] + t_c1[h] for h in heads]], np.float32)
        in_maps.append({
            "x": x[b],
            "xres": np.ascontiguousarray(x[b][:, cols]) + c1[None, cols],
            "biasb": bias[b],
            "wproj": wproj.astype(bf),
            "sct": sct,
        })
    res1 = run_bass_kernel_spmd(l1, in_maps, core_ids=list(range(8)),
                                trace=trace)
    if trace:
        LAST_EXEC_NS["l1"] = res1.exec_time_ns
    attn = np.empty((2, N, D), np.float32)
    for core in range(8):
        b, hh = core // 4, core % 4
        attn[b][:, hh * 128:(hh + 1) * 128] = res1.results[core]["outcols"]

    # ---- host weight folding, layer 2 ----
    W2g = (np.asarray(g2, np.float32)[:, None] * np.asarray(W2, np.float32))
    c2 = np.asarray(b2, np.float32) @ np.asarray(W2, np.float32)
    wa_s2 = W2g @ np.asarray(a_src2, np.float32)[0]                    # [D]
    wa_d2 = W2g @ np.asarray(a_dst2, np.float32)[0]
    s_c2 = float(c2 @ np.asarray(a_src2, np.float32)[0])
    t_c2 = float(c2 @ np.asarray(a_dst2, np.float32)[0])
    wproj2 = np.concatenate([W2g, wa_s2[:, None], wa_d2[:, None]], axis=1)

    in_maps2 = []
    for core in range(8):
        b, qc = core // 4, core % 4
        rows = slice(qc * 512, (qc + 1) * 512)
        in_maps2.append({
            "attnb": attn[b],
            "resid": attn[b][rows] + c2[None, :],
            "biasc": np.ascontiguousarray(bias[b][rows]),
            "wproj2": wproj2.astype(bf),
            "sct2": np.array([[s_c2 + t_c2]], np.float32),
        })
    res2 = run_bass_kernel_spmd(l2, in_maps2, core_ids=list(range(8)),
                                trace=trace)
    if trace:
        LAST_EXEC_NS["l2"] = res2.exec_time_ns

    out = np.empty((2, N, D), np.float32)
    for core in range(8):
        b, qc = core // 4, core % 4
        out[b][qc * 512:(qc + 1) * 512] = res2.results[core]["outc"]
    return out


# revision 41
# speedup vs baseline: 1.3130x; 1.3130x over previous
"""Fused 2-layer GAT block for Trainium2, SPMD over 8 NeuronCores.

Problem shapes (hardcoded): B=2, N=2048, D=512, H1=8 (Dh=64), H2=1 (Dh=512).

Strategy
--------
Two SPMD launches (layer 2 depends on the full layer-1 output):

Launch 1 (layer 1), core = (batch b, head-pair hh):  b = core//4, hh = core%4.
  Each core computes LN(x[b]) @ W1 (full, redundant within a batch group),
  then flash-style attention for its 2 heads over all 2048 queries:
  scores are built directly in transposed layout [128 keys_j, 2048 queries_i]
  (so the attn @ h matmul needs no per-tile transposes), the softmax
  denominator comes free from a ones-column appended to h (lhsT), and
  softmax normalization is folded into the output epilogue.
Launch 2 (layer 2), core = (batch b, query-chunk qc): 512 query rows/core,
  single head with Dh=512; same transposed-score scheme with 4+1 lhsT chunks.

Host-side weight folding: LN affine (g,b) folds into W' = diag(g)@W and
c = b@W; the attention projections a_src/a_dst fold into extra projection
columns wa = W'@a (so s,t come out of the projection matmul for free); the
constant c shifts s,t by scalars (passed as a tiny input) and the attention
output by +c (folded into the residual input on host).

The bias transpose pipeline (DMA column slab -> PE 128x128 transposes ->
PSUM -> copy-cast to bf16) is emitted first and runs in its own PSUM bank
pair so it overlaps LN/projection; scores overlap the tail of both.
"""

import os
import numpy as np

import concourse.bass as bass
import concourse.bacc as bacc
import concourse.mybir as mybir
import concourse.tile as tile
from concourse.bass_utils import run_bass_kernel_spmd
from concourse.masks import make_identity

F32 = mybir.dt.float32
BF16 = mybir.dt.bfloat16
AF = mybir.ActivationFunctionType
ALU = mybir.AluOpType

ALPHA = 0.2
EPS = 1e-6
P = 128
N = 2048
D = 512
NT = N // P          # 16 row tiles
KB = D // P          # 4 contraction blocks
JB = N // P          # 16 key blocks

# Engine-balance knobs (tuned on HW):
# leaky path: PRELU_NUM of every 16 score tiles use Prelu on ScalarE
PRELU_NUM = int(os.environ.get("GAT_PRELU_NUM", "8"))
# bias fp32->bf16 cast: every k-th chunk on DVE, rest on ScalarE
CAST_DVE_K = int(os.environ.get("GAT_CAST_DVE_K", "2"))
# biasT PSUM->SBUF copy: every k-th chunk on DVE, rest on ScalarE
COPY_DVE_K = int(os.environ.get("GAT_COPY_DVE_K", "2"))

LAST_EXEC_NS = {}
LAST_RES = {}


def _install_ntff_hook():
    """Wire up NTFF profiling under axon when antenv.axon_hooks is absent."""
    import sys
    import types
    try:
        from antenv.axon_hooks import get_axon_ntff_profile_hook  # noqa: F401
        return
    except ImportError:
        pass
    try:
        from trn_agent_boot.trn_boot import _ntff_profile_via_ctypes
        hook = _ntff_profile_via_ctypes("/opt/axon/libaxon_pjrt.so")
    except Exception:
        hook = None
    mod = types.ModuleType("antenv.axon_hooks")
    mod.get_axon_ntff_profile_hook = lambda: hook
    mod.set_axon_ntff_profile_hook = lambda h: None
    sys.modules["antenv.axon_hooks"] = mod
    import antenv
    antenv.axon_hooks = mod
    import concourse.bass_utils as bu
    bu.upload_artifacts = lambda tmpdir: tmpdir


def _bcast_ap(row_ap, parts=P):
    """DRAM row -> broadcast-over-partitions AP (partition stride 0)."""
    return bass.AP(
        tensor=row_ap.tensor,
        offset=row_ap.offset,
        ap=[[0, parts]] + [list(d) for d in row_ap.ap],
    )


def _ln_tile(nc, work, eps_t, x_ap, out_ap):
    """LayerNorm one [128, D] tile (fp32 stats) -> bf16 normalized tile."""
    stats = work.tile([P, 6], F32, tag="stats")
    mv = work.tile([P, 2], F32, tag="mv")
    nc.vector.bn_stats(stats, x_ap)
    nc.vector.bn_aggr(mv, stats)
    std = work.tile([P, 1], F32, tag="std")
    nc.scalar.activation(std, mv[:, 1:2], AF.Sqrt, bias=eps_t)
    rstd = work.tile([P, 1], F32, tag="rstd")
    nc.vector.reciprocal(rstd, std)
    nc.vector.tensor_scalar(out_ap, x_ap, mv[:, 0:1], rstd, ALU.subtract, ALU.mult)


def _bias_transpose(nc, tc, bd, braw_pool, ptB, biasT, id_bf, rows, jp,
                    after=None):
    """Two key-blocks of the bias: slab DMA, cast to bf16, PE-transpose (bf16),
    copy out of PSUM.  rows: number of query rows (free dim of biasT).

    Loading two 128-col blocks per DMA halves the 512B-descriptor count;
    HWDGE issue alternates sync/scalar; the fp32->bf16 cast runs on whichever
    of DVE/ACT has prologue slack; bf16 PE transposes are 4x faster than fp32.
    """
    import bass_rust
    nrt = rows // P
    braw = braw_pool.tile([P, nrt, 2, P], F32, tag="braw")
    eng = nc.sync if jp % 2 == 0 else nc.scalar
    dma = eng.dma_start(
        out=braw,
        in_=bd[:, jp * 2 * P:(jp + 1) * 2 * P]
        .rearrange("(nt p) (jl j) -> p nt jl j", p=P, j=P))
    if after is not None:
        bass_rust.add_dep_helper(dma.ins, after.ins,
                                 reason="bias slabs yield DMA BW to x")
    for jl in range(2):
        jb = 2 * jp + jl
        for cc in range(0, nrt, 8):
            nchunk = min(8, nrt - cc)
            bbf = braw_pool.tile([P, 8, P], BF16, tag="bbf")
            bsrc = braw[:, cc:cc + nchunk, jl]
            ci = jb * max(nrt // 8, 1) + cc // 8
            if ci % CAST_DVE_K == 0:
                nc.vector.tensor_copy(out=bbf[:, 0:nchunk], in_=bsrc)
            else:
                nc.scalar.copy(bbf[:, 0:nchunk], bsrc)
            pt = ptB.tile([P, 8, P], BF16, tag="ptB")
            for c in range(nchunk):
                nc.tensor.transpose(pt[:, c], bbf[:, c], id_bf)
            dst = biasT[:, jb, cc * P:(cc + nchunk) * P]
            src = pt[:, 0:nchunk].rearrange("p a b -> p (a b)")
            if ci % COPY_DVE_K == 0:
                nc.vector.tensor_copy(out=dst, in_=src)
            else:
                nc.scalar.copy(dst, src)


def _score_tile(nc, spool, sbcast_ap, t_col, at_col, biasT_ap, nfree, use_prelu):
    """exp(leaky(s_i + t_j) + bias_ji) as a [128, nfree] bf16 tile."""
    e = spool.tile([P, nfree], BF16, tag="s1")
    if use_prelu:
        l = spool.tile([P, nfree], BF16, tag="s3")
        nc.scalar.activation(l, sbcast_ap, AF.Prelu, bias=t_col, scale=1.0,
                             alpha=ALPHA)
        nc.vector.tensor_tensor(e, l, biasT_ap, ALU.add)
    else:
        u = spool.tile([P, nfree], BF16, tag="s1")
        nc.vector.tensor_scalar(u, sbcast_ap, t_col, None, ALU.add)
        v = spool.tile([P, nfree], BF16, tag="s2")
        nc.vector.tensor_scalar(v, sbcast_ap, ALPHA, at_col, ALU.mult, ALU.add)
        l = spool.tile([P, nfree], BF16, tag="s3")
        nc.vector.tensor_tensor(l, u, v, ALU.max)
        nc.vector.tensor_tensor(e, l, biasT_ap, ALU.add)
    p = spool.tile([P, nfree], BF16, tag="s2")
    nc.scalar.activation(p, e, AF.Exp)
    return p


def build_l1():
    """Layer-1 program: per core, 2 heads x 2048 queries x 2048 keys."""
    nc = bacc.Bacc("TRN2", target_bir_lowering=False, debug=False, num_devices=8)
    xd = nc.declare_dram_parameter("x", [N, D], F32, isOutput=False)
    xrd = nc.declare_dram_parameter("xres", [N, 2 * 64], F32, isOutput=False)
    bd = nc.declare_dram_parameter("biasb", [N, N], F32, isOutput=False)
    wd = nc.declare_dram_parameter("wproj", [D, 132], BF16, isOutput=False)
    sctd = nc.declare_dram_parameter("sct", [1, 2], F32, isOutput=False)
    od = nc.declare_dram_parameter("outcols", [N, 128], F32, isOutput=True)

    with tile.TileContext(nc) as tc:
        with (
            tc.tile_pool(name="const", bufs=1) as const,
            tc.tile_pool(name="big", bufs=1) as big,
            tc.tile_pool(name="work", bufs=4) as work,
            tc.tile_pool(name="braw", bufs=2) as brp,
            tc.tile_pool(name="ptB", bufs=2, space="PSUM") as ptB,
        ):
            id_bf = const.tile([P, P], BF16)
            id_f32 = const.tile([P, P], F32)
            eps_t = const.tile([P, 1], F32)
            nc.vector.memset(eps_t, EPS)
            w_sb = const.tile([P, KB, 132], BF16)
            nc.sync.dma_start(out=w_sb,
                              in_=wd[:].rearrange("(kb p) c -> p kb c", p=P))
            sct_sb = const.tile([P, 2], F32)
            nc.gpsimd.dma_start(out=sct_sb, in_=_bcast_ap(sctd[0, :]))

            xres_sb = big.tile([P, NT, 128], F32)
            nc.sync.dma_start(out=xres_sb,
                              in_=xrd[:].rearrange("(nt p) d -> p nt d", p=P))

            h_aug = big.tile([P, NT, 2, 65], BF16)
            nc.vector.memset(h_aug[:, :, :, 64:65], 1.0)
            st = big.tile([P, NT, 4], F32)       # proj cols: src0 src1 dst0 dst1
            t_adj = big.tile([P, NT, 2], F32)    # t + (s_c + t_c), per head
            at_adj = big.tile([P, NT, 2], F32)   # alpha * t_adj
            biasT = big.tile([P, JB, N], BF16)
            sbcast = big.tile([P, 2, N], BF16)
            out_stage = big.tile([P, NT, 128], F32)
            xnT = big.tile([P, KB, N], BF16)

            # ---- LN + transpose + projection (h, s, t) ----
            with tc.tile_pool(name="sbA", bufs=1) as sbA:
                xqs, xdmas = [], []
                for q in range(4):
                    xq = sbA.tile([P, 4, D], F32, tag="xq", bufs=3,
                                  name=f"xq{q}")
                    xdmas.append(nc.gpsimd.dma_start(
                        out=xq,
                        in_=xd[q * 512:(q + 1) * 512]
                        .rearrange("(nt p) d -> p nt d", p=P)))
                    xqs.append(xq)
                make_identity(nc, id_bf)
                make_identity(nc, id_f32)

                # bias transpose pipeline (independent) overlaps everything
                for jp in range(JB // 2):
                    _bias_transpose(nc, tc, bd, brp, ptB, biasT, id_bf, N, jp,
                                    after=xdmas[-1] if jp < 2 else None)

                with (
                    tc.tile_pool(name="ptA", bufs=3, space="PSUM") as ptA,
                    tc.tile_pool(name="ptS", bufs=1, space="PSUM") as ptS,
                ):
                    # pass 1: LN + transpose + the tiny s/t-column matmuls
                    for nt in range(NT):
                        xn = sbA.tile([P, D], BF16, tag="xn", bufs=3)
                        _ln_tile(nc, work, eps_t, xqs[nt // 4][:, nt % 4], xn)
                        ptt = ptA.tile([P, KB, P], BF16, tag="ptr")
                        for kb in range(KB):
                            nc.tensor.transpose(
                                ptt[:, kb], xn[:, kb * P:(kb + 1) * P], id_bf)
                        xnTs = xnT[:, :, nt * P:(nt + 1) * P]
                        if nt % 2 == 0:
                            nc.vector.tensor_copy(out=xnTs, in_=ptt)
                        else:
                            nc.scalar.copy(xnTs, ptt)
                        ptst = ptS.tile([P, 4], F32, tag="ppst")
                        for kb in range(KB):
                            nc.tensor.matmul(
                                ptst, xnT[:, kb, nt * P:(nt + 1) * P],
                                w_sb[:, kb, 128:132],
                                start=(kb == 0), stop=(kb == KB - 1))
                        nc.vector.tensor_copy(out=st[:, nt], in_=ptst)

                    # t columns (+ const shift), alpha*t
                    for hi in range(2):
                        nc.vector.tensor_scalar(
                            t_adj[:, :, hi], st[:, :, 2 + hi],
                            sct_sb[:, hi:hi + 1], None, ALU.add)
                    nc.vector.tensor_scalar(at_adj, t_adj, ALPHA, None, ALU.mult)

                    # s broadcast rows via transpose + tiny DMA round trip
                    with tc.tile_pool(name="sdram", bufs=1, space="DRAM") as sdram:
                        s_dram = sdram.tile([2, N], BF16)
                        for hi in range(2):
                            pts = ptS.tile([NT, P], F32, tag="pts")
                            nc.tensor.transpose(pts, st[:, :, hi], id_f32)
                            sfl = work.tile([NT, P], BF16, tag="sfl")
                            nc.scalar.copy(sfl, pts)
                            nc.sync.dma_start(
                                out=s_dram[hi].rearrange("(a b) -> a b", b=P),
                                in_=sfl)
                        for hi in range(2):
                            nc.gpsimd.dma_start(out=sbcast[:, hi],
                                                in_=_bcast_ap(s_dram[hi]))

            # ---- scores + attention matmul + epilogue ----
            with (
                tc.tile_pool(name="spool", bufs=3) as spool,
                tc.tile_pool(name="evp", bufs=1) as evp,
                tc.tile_pool(name="pacc", bufs=1, space="PSUM") as pacc,
                tc.tile_pool(name="ptro", bufs=2, space="PSUM") as ptrop,
            ):
                # heavy h projection — overlaps the early score tiles
                for nt in range(NT):
                    pth = ptrop.tile([P, 128], F32, tag="ptmp", name=f"pth{nt}")
                    for kb in range(KB):
                        nc.tensor.matmul(
                            pth, xnT[:, kb, nt * P:(nt + 1) * P],
                            w_sb[:, kb, 0:128],
                            start=(kb == 0), stop=(kb == KB - 1))
                    nc.vector.tensor_copy(
                        out=h_aug[:, nt, :, 0:64],
                        in_=pth.rearrange("p (h k) -> p h k", h=2))
                for hi in range(2):
                    acc = pacc.tile([65, N], F32, tag="acc")
                    for jb in range(JB):
                        p = _score_tile(
                            nc, spool, sbcast[:, hi],
                            t_adj[:, jb, hi:hi + 1], at_adj[:, jb, hi:hi + 1],
                            biasT[:, jb], N,
                            use_prelu=((jb * PRELU_NUM) % 16 < PRELU_NUM))
                        for ih in range(4):
                            sl = slice(ih * 512, (ih + 1) * 512)
                            nc.tensor.matmul(acc[:, sl], h_aug[:, jb, hi],
                                             p[:, sl], start=(jb == 0),
                                             stop=(jb == JB - 1))
                    # epilogue: copy out, transpose back, normalize, residual
                    ev = evp.tile([65, N], F32, tag="ev")
                    nc.vector.tensor_copy(out=ev, in_=acc)
                    ov = evp.tile([P, NT, 65], F32, tag="ov")
                    for cc in range(0, NT, 4):
                        ptr = ptrop.tile([P, 4, P], F32, tag="ptmp")
                        for c in range(4):
                            nc.tensor.transpose(
                                ptr[:, c, 0:65],
                                ev[:, (cc + c) * P:(cc + c + 1) * P],
                                id_f32[0:65, 0:65])
                        nc.scalar.copy(ov[:, cc:cc + 4], ptr[:, :, 0:65])
                    rz = work.tile([P, NT], F32, tag="rz")
                    nc.vector.reciprocal(rz, ov[:, :, 64])
                    for nt in range(NT):
                        sc = work.tile([P, 64], F32, tag="sc")
                        nc.vector.tensor_scalar(sc, ov[:, nt, 0:64],
                                                rz[:, nt:nt + 1], None, ALU.mult)
                        nc.vector.tensor_tensor(
                            out_stage[:, nt, hi * 64:(hi + 1) * 64], sc,
                            xres_sb[:, nt, hi * 64:(hi + 1) * 64], ALU.add)

            for q in range(4):
                eng = nc.sync if q % 2 == 0 else nc.scalar
                eng.dma_start(
                    out=od[q * 512:(q + 1) * 512]
                    .rearrange("(nt p) c -> p nt c", p=P),
                    in_=out_stage[:, q * 4:(q + 1) * 4])

    nc.compile()
    return nc


def build_l2():
    """Layer-2 program: per core, 1 head x 512 queries x 2048 keys."""
    nc = bacc.Bacc("TRN2", target_bir_lowering=False, debug=False, num_devices=8)
    ad = nc.declare_dram_parameter("attnb", [N, D], F32, isOutput=False)
    rd = nc.declare_dram_parameter("resid", [512, D], F32, isOutput=False)
    bd = nc.declare_dram_parameter("biasc", [512, N], F32, isOutput=False)
    wd = nc.declare_dram_parameter("wproj2", [D, 514], BF16, isOutput=False)
    sctd = nc.declare_dram_parameter("sct2", [1, 1], F32, isOutput=False)
    od = nc.declare_dram_parameter("outc", [512, D], F32, isOutput=True)

    IQ = 512            # queries per core
    QT = IQ // P        # 4 query tiles

    with tile.TileContext(nc) as tc:
        with (
            tc.tile_pool(name="const", bufs=1) as const,
            tc.tile_pool(name="big", bufs=1) as big,
            tc.tile_pool(name="work", bufs=4) as work,
            tc.tile_pool(name="braw", bufs=2) as brp,
        ):
            id_bf = const.tile([P, P], BF16)
            id_f32 = const.tile([P, P], F32)
            eps_t = const.tile([P, 1], F32)
            nc.vector.memset(eps_t, EPS)
            ones_col = const.tile([P, 1], BF16)
            nc.vector.memset(ones_col, 1.0)
            w_sb = const.tile([P, KB, 514], BF16)
            nc.sync.dma_start(out=w_sb,
                              in_=wd[:].rearrange("(kb p) c -> p kb c", p=P))
            sct_sb = const.tile([P, 1], F32)
            nc.gpsimd.dma_start(out=sct_sb, in_=_bcast_ap(sctd[0, :]))

            r_sb = big.tile([P, QT, D], F32)
            nc.sync.dma_start(out=r_sb,
                              in_=rd[:].rearrange("(nt p) d -> p nt d", p=P))

            h2 = big.tile([P, NT, D], BF16)
            sq = big.tile([P, QT], F32)
            t_adj = big.tile([P, NT, 1], F32)
            at_adj = big.tile([P, NT, 1], F32)
            biasT = big.tile([P, JB, IQ], BF16)
            sbcast = big.tile([P, IQ], BF16)
            out_stage = big.tile([P, QT, D], F32)
            xnT = big.tile([P, KB, N], BF16)

            # ---- LN (full batch for keys, query chunk for s) + proj ----
            with tc.tile_pool(name="sbA", bufs=1) as sbA:
                aqs, admas = [], []
                for q in range(4):
                    aq = sbA.tile([P, 4, D], F32, tag="aq", bufs=4,
                                  name=f"aq{q}")
                    admas.append(nc.gpsimd.dma_start(
                        out=aq,
                        in_=ad[q * 512:(q + 1) * 512]
                        .rearrange("(nt p) d -> p nt d", p=P)))
                    aqs.append(aq)
                adma = admas[-1]
                make_identity(nc, id_bf)
                make_identity(nc, id_f32)

                # bias transpose pipeline (independent) overlaps everything
                with tc.tile_pool(name="ptB", bufs=2, space="PSUM") as ptB:
                    for jp in range(JB // 2):
                        _bias_transpose(nc, tc, bd, brp, ptB, biasT, id_bf,
                                        IQ, jp, after=adma if jp < 2 else None)
                xnqT = sbA.tile([P, KB, IQ], BF16)
                with (
                    tc.tile_pool(name="ptA", bufs=3, space="PSUM") as ptA,
                    tc.tile_pool(name="ptS", bufs=1, space="PSUM") as ptS,
                ):
                    # s for the query chunk first (unblocks scores early)
                    for nt in range(QT):
                        xn = sbA.tile([P, D], BF16, tag="xn", bufs=3)
                        _ln_tile(nc, work, eps_t, r_sb[:, nt], xn)
                        ptt = ptA.tile([P, KB, P], BF16, tag="ptr")
                        for kb in range(KB):
                            nc.tensor.transpose(
                                ptt[:, kb], xn[:, kb * P:(kb + 1) * P], id_bf)
                        nc.vector.tensor_copy(
                            out=xnqT[:, :, nt * P:(nt + 1) * P], in_=ptt)
                        ptq2 = ptS.tile([P, 2], F32, tag="ppst")
                        for kb in range(KB):
                            nc.tensor.matmul(
                                ptq2, xnqT[:, kb, nt * P:(nt + 1) * P],
                                w_sb[:, kb, D:D + 2], start=(kb == 0),
                                stop=(kb == KB - 1))
                        nc.vector.tensor_copy(out=sq[:, nt:nt + 1],
                                              in_=ptq2[:, 0:1])

                    # s broadcast row
                    with tc.tile_pool(name="sdram", bufs=1, space="DRAM") as sdram:
                        s_dram = sdram.tile([1, IQ], BF16)
                        pts = ptS.tile([QT, P], F32, tag="pts")
                        nc.tensor.transpose(pts, sq, id_f32)
                        sfl = work.tile([QT, P], BF16, tag="sfl")
                        nc.scalar.copy(sfl, pts)
                        nc.sync.dma_start(
                            out=s_dram[0].rearrange("(a b) -> a b", b=P), in_=sfl)
                        nc.gpsimd.dma_start(out=sbcast, in_=_bcast_ap(s_dram[0]))

                    # keys: LN + transpose + t column
                    for nt in range(NT):
                        xn = sbA.tile([P, D], BF16, tag="xn", bufs=3)
                        _ln_tile(nc, work, eps_t, aqs[nt // 4][:, nt % 4], xn)
                        ptt = ptA.tile([P, KB, P], BF16, tag="ptr")
                        for kb in range(KB):
                            nc.tensor.transpose(
                                ptt[:, kb], xn[:, kb * P:(kb + 1) * P], id_bf)
                        xnTs = xnT[:, :, nt * P:(nt + 1) * P]
                        if nt % 2 == 0:
                            nc.vector.tensor_copy(out=xnTs, in_=ptt)
                        else:
                            nc.scalar.copy(xnTs, ptt)
                        ptst = ptS.tile([P, 2], F32, tag="ppst")
                        for kb in range(KB):
                            nc.tensor.matmul(
                                ptst, xnT[:, kb, nt * P:(nt + 1) * P],
                                w_sb[:, kb, D:D + 2],
                                start=(kb == 0), stop=(kb == KB - 1))
                        nc.vector.tensor_scalar(t_adj[:, nt], ptst[:, 1:2],
                                                sct_sb[:, 0:1], None, ALU.add)
                    nc.vector.tensor_scalar(at_adj, t_adj, ALPHA, None, ALU.mult)

            # ---- scores + attention matmul ----
            with (
                tc.tile_pool(name="spool", bufs=6) as spool,
                tc.tile_pool(name="pacc", bufs=1, space="PSUM") as pacc,
                tc.tile_pool(name="ptro", bufs=2, space="PSUM") as ptrop,
            ):
                # heavy h2 projection — overlaps the early score tiles
                for nt in range(NT):
                    pth = ptrop.tile([P, D], F32, tag="ptmp", name=f"pth{nt}")
                    for kb in range(KB):
                        nc.tensor.matmul(
                            pth, xnT[:, kb, nt * P:(nt + 1) * P],
                            w_sb[:, kb, 0:D],
                            start=(kb == 0), stop=(kb == KB - 1))
                    nc.vector.tensor_copy(out=h2[:, nt], in_=pth)
                accs = [pacc.tile([P, D], F32, tag=f"acc{m}", name=f"acc{m}")
                        for m in range(QT)]
                accz = pacc.tile([1, IQ], F32, tag="accz")
                for jb in range(JB):
                    p = _score_tile(
                        nc, spool, sbcast, t_adj[:, jb, 0:1], at_adj[:, jb, 0:1],
                        biasT[:, jb], IQ,
                        use_prelu=((jb * PRELU_NUM) % 16 < PRELU_NUM))
                    # lhsT = score chunks -> output lands as [queries, d]
                    for ic in range(QT):
                        nc.tensor.matmul(accs[ic], p[:, ic * P:(ic + 1) * P],
                                         h2[:, jb], start=(jb == 0),
                                         stop=(jb == JB - 1))
                    nc.tensor.matmul(accz, ones_col, p, start=(jb == 0),
                                     stop=(jb == JB - 1))

                # ---- epilogue: normalize + residual (no transposes) ----
                evz = work.tile([1, IQ], F32, tag="evz")
                nc.vector.tensor_copy(out=evz, in_=accz)
                ptz = pacc.tile([P, QT], F32, tag="ptz")
                for c in range(QT):
                    nc.tensor.transpose(ptz[:, c:c + 1],
                                        evz[0:1, c * P:(c + 1) * P],
                                        id_f32[0:1, 0:1])
                rz = work.tile([P, QT], F32, tag="rz")
                nc.vector.reciprocal(rz, ptz)

                for ic in range(QT):
                    sc = work.tile([P, D], F32, tag="sc")
                    nc.vector.tensor_scalar(sc, accs[ic], rz[:, ic:ic + 1],
                                            None, ALU.mult)
                    nc.vector.tensor_tensor(out_stage[:, ic], sc,
                                            r_sb[:, ic], ALU.add)

            for q in range(QT):
                eng = nc.sync if q % 2 == 0 else nc.scalar
                eng.dma_start(
                    out=od[q * P:(q + 1) * P].rearrange("(a p) c -> p a c", p=P),
                    in_=out_stage[:, q:q + 1])

    nc.compile()
    return nc


_CACHE = {}


def _get_programs():
    if "l1" not in _CACHE:
        _CACHE["l1"] = build_l1()
        _CACHE["l2"] = build_l2()
    return _CACHE["l1"], _CACHE["l2"]


def kernel(x, bias, W1, a_src1, a_dst1, g1, b1, W2, a_src2, a_dst2, g2, b2):
    x = np.asarray(x, np.float32)
    bias = np.asarray(bias, np.float32)
    bf = mybir.dt.np(BF16)
    trace = bool(os.environ.get("GAT_TRACE"))
    if trace:
        _install_ntff_hook()

    H, Dh = 8, 64
    # ---- host weight folding, layer 1 ----
    W1g = (np.asarray(g1, np.float32)[:, None] * np.asarray(W1, np.float32))
    c1 = np.asarray(b1, np.float32) @ np.asarray(W1, np.float32)      # [D]
    wa_s1 = np.einsum("dhk,hk->dh", W1g.reshape(D, H, Dh),
                      np.asarray(a_src1, np.float32))                  # [D, 8]
    wa_d1 = np.einsum("dhk,hk->dh", W1g.reshape(D, H, Dh),
                      np.asarray(a_dst1, np.float32))
    c1h = c1.reshape(H, Dh)
    s_c1 = (c1h * np.asarray(a_src1, np.float32)).sum(1)               # [8]
    t_c1 = (c1h * np.asarray(a_dst1, np.float32)).sum(1)

    l1, l2 = _get_programs()

    in_maps = []
    for core in range(8):
        b, hh = core // 4, core % 4
        cols = slice(hh * 128, (hh + 1) * 128)
        heads = [2 * hh, 2 * hh + 1]
        wproj = np.concatenate(
            [W1g[:, cols], wa_s1[:, heads], wa_d1[:, heads]], axis=1)
        sct = np.array([[s_c1[h] + t_c1[h] for h in heads]], np.float32)
        in_maps.append({
            "x": x[b],
            "xres": np.ascontiguousarray(x[b][:, cols]) + c1[None, cols],
            "biasb": bias[b],
            "wproj": wproj.astype(bf),
            "sct": sct,
        })
    res1 = run_bass_kernel_spmd(l1, in_maps, core_ids=list(range(8)),
                                trace=trace)
    if trace:
        LAST_EXEC_NS["l1"] = res1.exec_time_ns
        LAST_RES["l1"] = res1
    attn = np.empty((2, N, D), np.float32)
    for core in range(8):
        b, hh = core // 4, core % 4
        attn[b][:, hh * 128:(hh + 1) * 128] = res1.results[core]["outcols"]

    # ---- host weight folding, layer 2 ----
    W2g = (np.asarray(g2, np.float32)[:, None] * np.asarray(W2, np.float32))
    c2 = np.asarray(b2, np.float32) @ np.asarray(W2, np.float32)
    wa_s2 = W2g @ np.asarray(a_src2, np.float32)[0]                    # [D]
    wa_d2 = W2g @ np.asarray(a_dst2, np.float32)[0]
    s_c2 = float(c2 @ np.asarray(a_src2, np.float32)[0])
    t_c2 = float(c2 @ np.asarray(a_dst2, np.float32)[0])
    wproj2 = np.concatenate([W2g, wa_s2[:, None], wa_d2[:, None]], axis=1)

    in_maps2 = []
    for core in range(8):
        b, qc = core // 4, core % 4
        rows = slice(qc * 512, (qc + 1) * 512)
        in_maps2.append({
            "attnb": attn[b],
            "resid": attn[b][rows] + c2[None, :],
            "biasc": np.ascontiguousarray(bias[b][rows]),
            "wproj2": wproj2.astype(bf),
            "sct2": np.array([[s_c2 + t_c2]], np.float32),
        })
    res2 = run_bass_kernel_spmd(l2, in_maps2, core_ids=list(range(8)),
                                trace=trace)
    if trace:
        LAST_EXEC_NS["l2"] = res2.exec_time_ns
        LAST_RES["l2"] = res2

    out = np.empty((2, N, D), np.float32)
    for core in range(8):
        b, qc = core // 4, core % 4
        out[b][qc * 512:(qc + 1) * 512] = res2.results[core]["outc"]
    return out
